# revision 36
# baseline (speedup 1.0000x reference)
"""CTC loss kernel for Trainium2 (8 NeuronCores, data-parallel over batch).

Problem: nn_CTCLoss — B=4096, T=128, S=16, C=128, blank=0, zero_infinity,
reduction = mean(nll / S).

v5 pipeline (per core, 512 examples = 4 partition-blocks of 128):
  1. Host: targets = argmax(lable), pred cast to bf16 and pre-transposed to
     (c, blk, group, e, t) layout, one-hot gather matrix OH bf16, skip mask,
     identity.
  2. Per half-block (64 examples): plain DMA load of the (c, e, t) tile; one
     bf16 matmul per example (lhsT = x_e^T (c,t), rhs = OH_e (c,17)) gathers
     the 17 used channels -> (t, 17) PSUM; Act copies to gblk (t, ch, e) bf16.
  3. Per channel: PE transpose (bf16) -> (e, t) into paired PSUM tiles, exp
     with per-example scale bias (m fitted to the blank-channel mean growth
     rate) on the Act engine, two channels per activation op.
  4. CTC forward DP in the exp domain via the hardware scan instruction
     (state = (data0 + state) * data1), batch-on-partitions, wavefronting
     over the 33 extended-label slots; two block chains emitted interleaved
     so the DVE pipeline stays busy while Act/PE feed later blocks.
  5. nll[b] = -(log(A_31[T-1] + beta[T-1]) + T*m[b]); host does the
     zero_infinity masking and the mean.
"""

import sys
import numpy as np

sys.path.insert(0, "/opt/trn_rl_repo")

# ---- problem constants (hardcoded per contract) ----
B, T, C, S = 4096, 128, 128, 16
NCORES = 8
BC = B // NCORES          # 512 examples per core
NBLK = BC // 128          # 4 partition-blocks per core
NCH = S + 1               # 17 used channels: blank + 16 targets
# growth-rate estimator m[b] = M_A + M_B * mean_t(logp[b,:,0]) (fit offline)
M_A = 0.86674847
M_B = 0.36057915

_CACHE = {}


def _build_program():
    import concourse.bass as bass
    import concourse.tile as tile
    from concourse import bacc, mybir

    f32 = mybir.dt.float32
    bf16 = mybir.dt.bfloat16
    DPT = f32    # DP dtype (bf16 measured identical scan speed; keep f32)
    AOP = mybir.AluOpType
    AF = mybir.ActivationFunctionType

    nc = bacc.Bacc("TRN2", target_bir_lowering=False, debug=False)
    # pred pre-transposed on host: [c, blk, half, e_local(64), t]
    pred_h = nc.declare_dram_parameter("pred", [128, NBLK, 2, 64, T], bf16,
                                       isOutput=False)
    oh_h = nc.declare_dram_parameter("oh", [128, NBLK, 128 * NCH], bf16,
                                     isOutput=False)
    skv_h = nc.declare_dram_parameter("skv", [128, NBLK * S], f32, isOutput=False)
    idn_h = nc.declare_dram_parameter("idn", [128, 128], bf16, isOutput=False)
    out_h = nc.declare_dram_parameter("out", [128, NBLK], f32, isOutput=True)

    with tile.TileContext(nc) as tc:
        with (
            tc.tile_pool(name="const", bufs=1) as constp,
            tc.tile_pool(name="xt", bufs=3) as xtp,
            tc.tile_pool(name="gblk", bufs=2) as gblkp,
            tc.tile_pool(name="pb", bufs=4) as pbp,
            tc.tile_pool(name="ps", bufs=4) as psp,
            tc.tile_pool(name="abuf", bufs=4) as abufp,
            tc.tile_pool(name="w", bufs=4) as wp,
            tc.tile_pool(name="sc", bufs=8) as scp,
            tc.tile_pool(name="fin", bufs=1) as finp,
            tc.tile_pool(name="gps", bufs=2, space="PSUM") as g_psum,
            tc.tile_pool(name="pps", bufs=2, space="PSUM") as p_psum,
        ):
            # ---- constants ----
            ident = constp.tile([128, 128], bf16)
            nc.sync.dma_start(ident[:], idn_h[:])
            skv_sb = constp.tile([128, NBLK * S], f32)
            nc.sync.dma_start(skv_sb[:], skv_h[:])
            oh_sb = constp.tile([128, NBLK, 128, NCH], bf16)

            zeros = constp.tile([128, 128], DPT)    # zero source for slot 0
            nc.gpsimd.memset(zeros[:], 0.0)

            y_all = finp.tile([128, NBLK], f32)
            m128_all = finp.tile([128, NBLK], f32)

            # ---------- phase A: load + gather + exp for one block ----------
            def phase_a(blk, split_first=False):
                nc.scalar.dma_start(
                    oh_sb[:, blk],
                    oh_h[:, blk].rearrange("p (e s) -> p e s", e=128))
                if split_first:
                    # preload the Exp/Ln activation tables during DMA warmup
                    warm = scp.tile([128, 1], f32)
                    nc.gpsimd.memset(warm[:], 1.0)
                    warm2 = scp.tile([128, 1], f32)
                    nc.scalar.activation(warm2[:], warm[:], AF.Exp)
                    nc.scalar.activation(warm2[:], warm[:], AF.Ln)
                gblk = gblkp.tile([128, NCH, 128], bf16)  # (t, ch, e)
                for h in range(2):                        # half-blocks of 64
                    xt = xtp.tile([128, 64, 128], bf16)   # (c, e, t)
                    if split_first and h == 0:
                        # quarter the very first load so the PE starts early
                        for q in range(4):
                            nc.sync.dma_start(
                                xt[:, q * 16:(q + 1) * 16],
                                pred_h[:, blk, h].rearrange(
                                    "p (q e) t -> p q e t", q=4)[:, q])
                    else:
                        nc.sync.dma_start(xt[:], pred_h[:, blk, h])
                    for q in range(4):                    # 16 examples each
                        gps = g_psum.tile([128, 16, NCH], f32)  # (t, e, ch)
                        for e in range(16):
                            eg = h * 64 + q * 16 + e
                            nc.tensor.matmul(gps[:, e], xt[:, q * 16 + e],
                                             oh_sb[:, blk, eg],
                                             start=True, stop=True)
                        c0 = h * 64 + q * 16
                        nc.scalar.copy(gblk[:, :, c0:c0 + 16],
                                       gps[:].rearrange("t e c -> t c e"))

                # ---- channel transposes + exp (+ per-example scale) ----
                pps = p_psum.tile([128, 128], bf16)
                nc.tensor.transpose(pps[:], gblk[:, 0], ident[:])  # blank ch
                # blank-channel row sum via the Act accumulator
                mraw = scp.tile([128, 1], f32)
                scratch = wp.tile([128, 128], bf16)
                nc.scalar.activation(scratch[:], pps[:], AF.Copy,
                                     accum_out=mraw[:])
                bias_blk = scp.tile([128, 1], f32)
                nc.scalar.activation(bias_blk[:], mraw[:], AF.Copy,
                                     bias=-M_A, scale=-M_B / T)
                nc.scalar.activation(m128_all[:, blk:blk + 1], mraw[:], AF.Copy,
                                     bias=-float(T) * M_A, scale=-M_B)
                pb = pbp.tile([128, 128], DPT)
                nc.scalar.activation(pb[:], pps[:], AF.Exp, bias=bias_blk[:])

                ps = psp.tile([128, S, 128], DPT)
                for s2 in range(S // 2):                   # channels in pairs
                    pp2 = p_psum.tile([128, 2, 128], bf16)
                    nc.tensor.transpose(pp2[:, 0], gblk[:, 2 * s2 + 1], ident[:])
                    nc.tensor.transpose(pp2[:, 1], gblk[:, 2 * s2 + 2], ident[:])
                    nc.scalar.activation(ps[:, 2 * s2:2 * s2 + 2], pp2[:],
                                         AF.Exp, bias=bias_blk[:])
                return pb, ps

            # ---------- phase B: the CTC DP for one block ----------
            # generator of steps so two blocks can be emitted interleaved
            def phase_b(blk, pb, ps):
                abuf = abufp.tile([128, 4 * 129], DPT)
                nc.gpsimd.memset(
                    abuf[:].rearrange("p (r t) -> p r t", r=4)[:, :, 0:1], 0.0)

                def reg(l):
                    return (l % 4) * 129

                def shA(l):  # A_l shifted by one step in t (guard col leads)
                    return abuf[:, reg(l):reg(l) + 128]

                # CTC update maps exactly onto the scan instruction:
                #   state = (data0[t] + state) * data1[t]
                def scan(l, u_ap, p_ap, initial=0.0):
                    nc.vector.tensor_tensor_scan(
                        abuf[:, reg(l) + 1:reg(l) + 129], u_ap, p_ap,
                        initial=initial, op0=AOP.add, op1=AOP.mult)

                # t=0 injections enter through the scan initial state:
                # alpha_l[0] = (data0[0] + initial) * P[0] with data0[0] = 0
                scan(0, zeros[:], pb[:], initial=1.0)
                yield
                scan(1, shA(0), ps[:, 0], initial=1.0)
                yield
                for l in range(2, 2 * S):
                    if l % 2 == 0:
                        scan(l, shA(l - 1), pb[:])
                    else:
                        s = (l - 1) // 2
                        w = wp.tile([128, 128], DPT)
                        nc.vector.scalar_tensor_tensor(
                            w[:], shA(l - 2),
                            skv_sb[:, blk * S + s:blk * S + s + 1], shA(l - 1),
                            op0=AOP.mult, op1=AOP.add)
                        scan(l, w[:], ps[:, s])
                    yield
                # beta scan (slot 32, last blank) into region of l=32
                scan(32, shA(31), pb[:])
                yield
                # y = A_31[T-1] + beta[T-1]
                nc.gpsimd.tensor_tensor(y_all[:, blk:blk + 1],
                                        abuf[:, reg(31) + 128:reg(31) + 129],
                                        abuf[:, reg(32) + 128:reg(32) + 129],
                                        op=AOP.add)

            def run_pair(specs):
                gens = [phase_b(blk, pb, ps) for blk, pb, ps in specs]
                done = [False] * len(gens)
                while not all(done):
                    for i, gen in enumerate(gens):
                        if not done[i]:
                            try:
                                next(gen)
                            except StopIteration:
                                done[i] = True

            # software pipeline: A0 A1 | B01 (emitted) A2 A3 | B23
            pb0, ps0 = phase_a(0, split_first=True)
            pb1, ps1 = phase_a(1)
            run_pair([(0, pb0, ps0), (1, pb1, ps1)])
            pb2, ps2 = phase_a(2)
            pb3, ps3 = phase_a(3)
            run_pair([(2, pb2, ps2), (3, pb3, ps3)])

            # ---- finalize: nll = -(log y + T*m) ----
            logy = finp.tile([128, NBLK], f32)
            nc.scalar.activation(logy[:], y_all[:], AF.Ln)
            nll = finp.tile([128, NBLK], f32)
            nc.vector.scalar_tensor_tensor(nll[:], logy[:], -1.0, m128_all[:],
                                           op0=AOP.mult, op1=AOP.add)
            nc.sync.dma_start(out_h[:], nll[:])

    nc.finalize()
    return nc


def _host_prep(prediction, lable):
    """Per-core input maps from full inputs."""
    import ml_dtypes
    bf = ml_dtypes.bfloat16
    tg = np.argmax(lable, axis=-1).astype(np.int64)        # (B, S)
    # skip allowed at odd slot l=2s+1 (s>=1) iff tg_s != tg_{s-1}
    skv = np.zeros((B, S), dtype=np.float32)
    skv[:, 1:] = (tg[:, 1:] != tg[:, :-1]).astype(np.float32)

    # one-hot gather matrix: oh[c, b_local, s] = 1 iff channel s of example
    # b_local selects class c (s=0 -> blank=0, s>=1 -> tg[b, s-1])
    oh = np.zeros((NCORES, 128, BC, NCH), dtype=bf)
    oh[:, 0, :, 0] = 1.0
    bidx = np.arange(B)
    core_i = bidx // BC
    loc_i = bidx % BC
    for s in range(S):
        oh[core_i, tg[:, s], loc_i, s + 1] = 1.0

    idn = np.eye(128, dtype=bf)

    in_maps = []
    for k in range(NCORES):
        sl = slice(k * BC, (k + 1) * BC)
        # skv layout: [partition p, blk*S + s] with example = blk*128 + p
        sk_k = np.ascontiguousarray(
            skv[sl].reshape(NBLK, 128, S).transpose(1, 0, 2).reshape(128, NBLK * S))
        # pred pre-transposed to [c, blk, half, e, t] (bf16)
        pk = prediction[sl].astype(bf).reshape(NBLK, 2, 64, T, C)
        pk = np.ascontiguousarray(pk.transpose(4, 0, 1, 2, 3))
        in_maps.append({
            "pred": pk,
            "oh": np.ascontiguousarray(oh[k].reshape(128, NBLK, 128 * NCH)),
            "skv": sk_k,
            "idn": idn,
        })
    return in_maps


def _combine(results):
    # out[core] is (128, NBLK): nll for example core*BC + blk*128 + p
    nll = np.stack([np.asarray(r["out"]) for r in results])   # (8, 128, 4)
    nll = nll.transpose(0, 2, 1).reshape(B)
    loss = np.where(np.isfinite(nll), nll, 0.0)
    return np.float32(np.mean(loss / np.float64(S)))


def kernel(prediction, lable):
    from concourse.bass_utils import run_bass_kernel_spmd

    prediction = np.asarray(prediction, dtype=np.float32)
    lable = np.asarray(lable, dtype=np.float32)
    if "nc" not in _CACHE:
        _CACHE["nc"] = _build_program()
    in_maps = _host_prep(prediction, lable)
    res = run_bass_kernel_spmd(_CACHE["nc"], in_maps, list(range(NCORES)))
    return _combine(res.results)


if __name__ == "__main__":
    rng = np.random.default_rng(0)
    p = rng.standard_normal((B, T, C), dtype=np.float32)
    l = rng.standard_normal((B, S, C), dtype=np.float32)
    print(kernel(p, l))


# revision 43
# speedup vs baseline: 1.0593x; 1.0593x over previous
"""CTC loss kernel for Trainium2 (8 NeuronCores, data-parallel over batch).

Problem: nn_CTCLoss — B=4096, T=128, S=16, C=128, blank=0, zero_infinity,
reduction = mean(nll / S).

v5 pipeline (per core, 512 examples = 4 partition-blocks of 128):
  1. Host: targets = argmax(lable), pred cast to bf16 and pre-transposed to
     (c, blk, group, e, t) layout, one-hot gather matrix OH bf16, skip mask,
     identity.
  2. Per half-block (64 examples): plain DMA load of the (c, e, t) tile; one
     bf16 matmul per example (lhsT = x_e^T (c,t), rhs = OH_e (c,17)) gathers
     the 17 used channels -> (t, 17) PSUM; Act copies to gblk (t, ch, e) bf16.
  3. Per channel: PE transpose (bf16) -> (e, t) into paired PSUM tiles, exp
     with per-example scale bias (m fitted to the blank-channel mean growth
     rate) on the Act engine, two channels per activation op.
  4. CTC forward DP in the exp domain via the hardware scan instruction
     (state = (data0 + state) * data1), batch-on-partitions, wavefronting
     over the 33 extended-label slots; two block chains emitted interleaved
     so the DVE pipeline stays busy while Act/PE feed later blocks.
  5. nll[b] = -(log(A_31[T-1] + beta[T-1]) + T*m[b]); host does the
     zero_infinity masking and the mean.
"""

import sys
import numpy as np

sys.path.insert(0, "/opt/trn_rl_repo")

# ---- problem constants (hardcoded per contract) ----
B, T, C, S = 4096, 128, 128, 16
NCORES = 8
BC = B // NCORES          # 512 examples per core
NBLK = BC // 128          # 4 partition-blocks per core
NCH = S + 1               # 17 used channels: blank + 16 targets
# growth-rate estimator m[b] = M_A + M_B * mean_t(logp[b,:,0]) (fit offline)
M_A = 0.86674847
M_B = 0.36057915

_CACHE = {}


def _build_program():
    import concourse.bass as bass
    import concourse.tile as tile
    from concourse import bacc, mybir

    f32 = mybir.dt.float32
    bf16 = mybir.dt.bfloat16
    DPT = f32    # DP dtype (bf16 measured identical scan speed; keep f32)
    AOP = mybir.AluOpType
    AF = mybir.ActivationFunctionType

    nc = bacc.Bacc("TRN2", target_bir_lowering=False, debug=False)
    # pred pre-transposed on host: [c, blk, half, e_local(64), t]
    pred_h = nc.declare_dram_parameter("pred", [128, NBLK, 2, 64, T], bf16,
                                       isOutput=False)
    oh_h = nc.declare_dram_parameter("oh", [128, NBLK, 128 * NCH], bf16,
                                     isOutput=False)
    skv_h = nc.declare_dram_parameter("skv", [128, NBLK * S], f32, isOutput=False)
    idn_h = nc.declare_dram_parameter("idn", [128, 128], bf16, isOutput=False)
    out_h = nc.declare_dram_parameter("out", [128, NBLK], f32, isOutput=True)

    with tile.TileContext(nc) as tc:
        with (
            tc.tile_pool(name="const", bufs=1) as constp,
            tc.tile_pool(name="xt", bufs=3) as xtp,
            tc.tile_pool(name="gblk", bufs=2) as gblkp,
            tc.tile_pool(name="pb", bufs=4) as pbp,
            tc.tile_pool(name="ps", bufs=4) as psp,
            tc.tile_pool(name="abuf", bufs=4) as abufp,
            tc.tile_pool(name="w", bufs=4) as wp,
            tc.tile_pool(name="sc", bufs=8) as scp,
            tc.tile_pool(name="fin", bufs=1) as finp,
            tc.tile_pool(name="gps", bufs=2, space="PSUM") as g_psum,
            tc.tile_pool(name="pps", bufs=2, space="PSUM") as p_psum,
        ):
            # ---- constants ----
            ident = constp.tile([128, 128], bf16)
            nc.sync.dma_start(ident[:], idn_h[:])
            skv_sb = constp.tile([128, NBLK * S], f32)
            nc.sync.dma_start(skv_sb[:], skv_h[:])
            oh_sb = constp.tile([128, NBLK, 128, NCH], bf16)

            zeros = constp.tile([128, 128], DPT)    # zero source for slot 0
            nc.gpsimd.memset(zeros[:], 0.0)
            ones_col = constp.tile([128, 1], bf16)  # blank row-sum reducer
            nc.gpsimd.memset(ones_col[:], 1.0)

            y_all = finp.tile([128, NBLK], f32)
            m128_all = finp.tile([128, NBLK], f32)

            # ---------- phase A: load + gather + exp for one block ----------
            def phase_a(blk, split_first=False):
                nc.sync.dma_start(
                    oh_sb[:, blk],
                    oh_h[:, blk].rearrange("p (e s) -> p e s", e=128))
                if split_first:
                    # preload the Exp/Ln activation tables during DMA warmup
                    warm = scp.tile([128, 1], f32)
                    nc.gpsimd.memset(warm[:], 1.0)
                    warm2 = scp.tile([128, 1], f32)
                    nc.scalar.activation(warm2[:], warm[:], AF.Exp)
                    nc.scalar.activation(warm2[:], warm[:], AF.Ln)
                gblk = gblkp.tile([128, NCH, 128], bf16)  # (t, ch, e)
                for h in range(2):                        # half-blocks of 64
                    xt = xtp.tile([128, 64, 128], bf16)   # (c, e, t)
                    if split_first and h == 0:
                        # quarter the very first load so the PE starts early
                        for q in range(4):
                            nc.sync.dma_start(
                                xt[:, q * 16:(q + 1) * 16],
                                pred_h[:, blk, h].rearrange(
                                    "p (q e) t -> p q e t", q=4)[:, q])
                    else:
                        nc.sync.dma_start(xt[:], pred_h[:, blk, h])
                    for q in range(4):                    # 16 examples each
                        gps = g_psum.tile([128, 16, NCH], f32)  # (t, e, ch)
                        for e in range(16):
                            eg = h * 64 + q * 16 + e
                            nc.tensor.matmul(gps[:, e], xt[:, q * 16 + e],
                                             oh_sb[:, blk, eg],
                                             start=True, stop=True)
                        c0 = h * 64 + q * 16
                        nc.scalar.copy(gblk[:, :, c0:c0 + 16],
                                       gps[:].rearrange("t e c -> t c e"))

                # ---- channel transposes + exp (+ per-example scale) ----
                pps = p_psum.tile([128, 128], bf16)
                nc.tensor.transpose(pps[:], gblk[:, 0], ident[:])  # blank ch
                # blank-channel row sum via the Act accumulator
                mraw = scp.tile([128, 1], f32)
                scratch = wp.tile([128, 128], bf16)
                nc.scalar.activation(scratch[:], pps[:], AF.Copy,
                                     accum_out=mraw[:])
                bias_blk = scp.tile([128, 1], f32)
                nc.scalar.activation(bias_blk[:], mraw[:], AF.Copy,
                                     bias=-M_A, scale=-M_B / T)
                nc.scalar.activation(m128_all[:, blk:blk + 1], mraw[:], AF.Copy,
                                     bias=-float(T) * M_A, scale=-M_B)
                pb = pbp.tile([128, 128], DPT)
                nc.scalar.activation(pb[:], pps[:], AF.Exp, bias=bias_blk[:])

                ps = psp.tile([128, S, 128], DPT)
                for s2 in range(S // 2):                   # channels in pairs
                    pp2 = p_psum.tile([128, 2, 128], bf16)
                    nc.tensor.transpose(pp2[:, 0], gblk[:, 2 * s2 + 1], ident[:])
                    nc.tensor.transpose(pp2[:, 1], gblk[:, 2 * s2 + 2], ident[:])
                    nc.scalar.activation(ps[:, 2 * s2:2 * s2 + 2], pp2[:],
                                         AF.Exp, bias=bias_blk[:])
                return pb, ps

            # ---------- phase B: the CTC DP for one block ----------
            # generator of steps so two blocks can be emitted interleaved
            def phase_b(blk, pb, ps):
                abuf = abufp.tile([128, 4 * 129], DPT)
                nc.gpsimd.memset(
                    abuf[:].rearrange("p (r t) -> p r t", r=4)[:, :, 0:1], 0.0)

                def reg(l):
                    return (l % 4) * 129

                def shA(l):  # A_l shifted by one step in t (guard col leads)
                    return abuf[:, reg(l):reg(l) + 128]

                # CTC update maps exactly onto the scan instruction:
                #   state = (data0[t] + state) * data1[t]
                def scan(l, u_ap, p_ap, initial=0.0):
                    nc.vector.tensor_tensor_scan(
                        abuf[:, reg(l) + 1:reg(l) + 129], u_ap, p_ap,
                        initial=initial, op0=AOP.add, op1=AOP.mult)

                # t=0 injections enter through the scan initial state:
                # alpha_l[0] = (data0[0] + initial) * P[0] with data0[0] = 0
                scan(0, zeros[:], pb[:], initial=1.0)
                yield
                scan(1, shA(0), ps[:, 0], initial=1.0)
                yield
                for l in range(2, 2 * S):
                    if l % 2 == 0:
                        scan(l, shA(l - 1), pb[:])
                    else:
                        s = (l - 1) // 2
                        w = wp.tile([128, 128], DPT)
                        nc.vector.scalar_tensor_tensor(
                            w[:], shA(l - 2),
                            skv_sb[:, blk * S + s:blk * S + s + 1], shA(l - 1),
                            op0=AOP.mult, op1=AOP.add)
                        scan(l, w[:], ps[:, s])
                    yield
                # beta scan (slot 32, last blank) into region of l=32
                scan(32, shA(31), pb[:])
                yield
                # y = A_31[T-1] + beta[T-1]
                nc.gpsimd.tensor_tensor(y_all[:, blk:blk + 1],
                                        abuf[:, reg(31) + 128:reg(31) + 129],
                                        abuf[:, reg(32) + 128:reg(32) + 129],
                                        op=AOP.add)

            def run_pair(specs):
                gens = [phase_b(blk, pb, ps) for blk, pb, ps in specs]
                done = [False] * len(gens)
                while not all(done):
                    for i, gen in enumerate(gens):
                        if not done[i]:
                            try:
                                next(gen)
                            except StopIteration:
                                done[i] = True

            # software pipeline: A0 A1 | B01 (emitted) A2 A3 | B23
            pb0, ps0 = phase_a(0, split_first=True)
            pb1, ps1 = phase_a(1)
            run_pair([(0, pb0, ps0), (1, pb1, ps1)])
            pb2, ps2 = phase_a(2)
            pb3, ps3 = phase_a(3)
            run_pair([(2, pb2, ps2), (3, pb3, ps3)])

            # ---- finalize: nll = -(log y + T*m) ----
            logy = finp.tile([128, NBLK], f32)
            nc.scalar.activation(logy[:], y_all[:], AF.Ln)
            nll = finp.tile([128, NBLK], f32)
            nc.vector.scalar_tensor_tensor(nll[:], logy[:], -1.0, m128_all[:],
                                           op0=AOP.mult, op1=AOP.add)
            nc.sync.dma_start(out_h[:], nll[:])

    nc.finalize()
    return nc


def _host_prep(prediction, lable):
    """Per-core input maps from full inputs."""
    import ml_dtypes
    bf = ml_dtypes.bfloat16
    tg = np.argmax(lable, axis=-1).astype(np.int64)        # (B, S)
    # skip allowed at odd slot l=2s+1 (s>=1) iff tg_s != tg_{s-1}
    skv = np.zeros((B, S), dtype=np.float32)
    skv[:, 1:] = (tg[:, 1:] != tg[:, :-1]).astype(np.float32)

    # one-hot gather matrix: oh[c, b_local, s] = 1 iff channel s of example
    # b_local selects class c (s=0 -> blank=0, s>=1 -> tg[b, s-1])
    oh = np.zeros((NCORES, 128, BC, NCH), dtype=bf)
    oh[:, 0, :, 0] = 1.0
    bidx = np.arange(B)
    core_i = bidx // BC
    loc_i = bidx % BC
    for s in range(S):
        oh[core_i, tg[:, s], loc_i, s + 1] = 1.0

    idn = np.eye(128, dtype=bf)

    in_maps = []
    for k in range(NCORES):
        sl = slice(k * BC, (k + 1) * BC)
        # skv layout: [partition p, blk*S + s] with example = blk*128 + p
        sk_k = np.ascontiguousarray(
            skv[sl].reshape(NBLK, 128, S).transpose(1, 0, 2).reshape(128, NBLK * S))
        # pred pre-transposed to [c, blk, half, e, t] (bf16)
        pk = prediction[sl].astype(bf).reshape(NBLK, 2, 64, T, C)
        pk = np.ascontiguousarray(pk.transpose(4, 0, 1, 2, 3))
        in_maps.append({
            "pred": pk,
            "oh": np.ascontiguousarray(oh[k].reshape(128, NBLK, 128 * NCH)),
            "skv": sk_k,
            "idn": idn,
        })
    return in_maps


def _combine(results):
    # out[core] is (128, NBLK): nll for example core*BC + blk*128 + p
    nll = np.stack([np.asarray(r["out"]) for r in results])   # (8, 128, 4)
    nll = nll.transpose(0, 2, 1).reshape(B)
    loss = np.where(np.isfinite(nll), nll, 0.0)
    return np.float32(np.mean(loss / np.float64(S)))


def kernel(prediction, lable):
    from concourse.bass_utils import run_bass_kernel_spmd

    prediction = np.asarray(prediction, dtype=np.float32)
    lable = np.asarray(lable, dtype=np.float32)
    if "nc" not in _CACHE:
        _CACHE["nc"] = _build_program()
    in_maps = _host_prep(prediction, lable)
    res = run_bass_kernel_spmd(_CACHE["nc"], in_maps, list(range(NCORES)))
    return _combine(res.results)


if __name__ == "__main__":
    rng = np.random.default_rng(0)
    p = rng.standard_normal((B, T, C), dtype=np.float32)
    l = rng.standard_normal((B, S, C), dtype=np.float32)
    print(kernel(p, l))
